# revision 30
# baseline (speedup 1.0000x reference)
"""Grouped-experts MoE FFN (SwiGLU) kernel for Trainium2, 8 NeuronCores.

Strategy: expert-parallel with host-side packing.  Token counts per expert
are data, so the host chops each expert's contiguous token block into
chunks and bins them into a uniform per-core "slot" structure
(S slots per core, compile-time sizes).  Every slot is bound to exactly
one expert; the expert's (host-pre-permuted) weights are plain kernel
inputs, so the SPMD program is identical on all 8 cores and needs no
device-side transposes or gather/scatter.

All matmul operands are bf16 (fp32 PSUM accumulation): LDWEIGHTS drops to
128 cycles and hides fully under >=256-row matmuls, and DMA bytes halve.

Per-core device program, per slot of capacity L (tokens):
  phase A: for each 128-row h-chunk (22 of them):
      psum1[128,L] = sum_dc w1r[hc,dc].T @ xT[dc]
      psum3[128,L] = sum_dc w3r[hc,dc].T @ xT[dc]
      h[hc] = silu(psum1) * psum3                     (ACT + DVE)
    (w2 tiles for phase B are DMA'd interleaved into this loop so the
     in-order DMA queue has them resident before phase B starts)
  phase B: for each 128-row d-chunk (8):
      po[128,L] = sum_hc w2r[dc,hc].T @ h[hc]
      DMA po -> outT[dc] (bf16, [D,L] layout)

Host then transposes each slot's [D, L] output back and scatters into the
full [T, D] result (padding rows stay zero).
"""

import itertools
import numpy as np
from functools import lru_cache

E, D, H, T = 8, 1024, 2816, 16384
P = 128
DC, HC = D // P, H // P  # 8, 22
NCORES = 8
NSLOTS = 3

_FALLBACK = (1024, 1024, 1024)  # feasible for any counts with sum <= T


def _try_assign(Ls, counts, budget=None):
    """Bounded DFS with memoized failures: for each expert pick chunk counts
    (n per size class) so every class uses <= NCORES slots.  Returns
    per-expert (n1..nS) or None (infeasible or node budget exhausted).
    `budget` is a single-element list decremented across calls."""
    S = len(Ls)
    if budget is None:
        budget = [1 << 30]
    # process big experts first: they constrain the search the most
    order = sorted(range(len(counts)), key=lambda e: -counts[e])
    gs = [counts[e] for e in order]

    # per-expert option lists (ns tuples covering g), waste-ascending,
    # minimal (no droppable chunk)
    def options(g):
        opts = []
        for ns_head in itertools.product(*([range(NCORES + 1)] * (S - 1))):
            cap_head = sum(n * L for n, L in zip(ns_head, Ls[:-1]))
            rem = g - cap_head
            n_last = max(0, -(-rem // Ls[-1]))
            if n_last > NCORES:
                continue
            ns = ns_head + (n_last,)
            cap = cap_head + n_last * Ls[-1]
            # minimality: no chunk entirely wasted
            if any(n > 0 and cap - Ls[k] >= g for k, n in enumerate(ns)):
                continue
            opts.append((cap - g, ns))
        opts.sort()
        return [ns for _, ns in opts]

    opt_lists = [options(g) for g in gs]
    failed = set()

    def dfs(i, used):
        if i == len(gs):
            return ()
        if (i, used) in failed:
            return None
        budget[0] -= 1
        if budget[0] < 0:
            return None
        for ns in opt_lists[i]:
            if any(u + n > NCORES for u, n in zip(used, ns)):
                continue
            sub = dfs(i + 1, tuple(u + n for u, n in zip(used, ns)))
            if sub is not None:
                return (ns,) + sub
        failed.add((i, used))
        return None

    res = dfs(0, (0,) * S)
    if res is None:
        return None
    # un-permute back to original expert order
    asg = [None] * len(counts)
    for pos, e in enumerate(order):
        asg[e] = res[pos]
    return tuple(asg)


@lru_cache(maxsize=None)
def _find_structure(counts):
    """Pick slot sizes minimizing capacity; classes returned largest-first
    so the (small) tail slot finishes last."""
    counts = list(counts)
    # slot sizes chunk into 512-col matmuls plus one trailing 256..448, so
    # every chunk stays inside a 2KB PSUM bank and is >=256 wide.
    sizes = [s for s in range(256, 1025, 64) if s % 512 in (0, 256, 320, 384, 448)]
    cands = sorted(
        itertools.combinations_with_replacement(sizes, NSLOTS),
        key=lambda Ls: (sum(Ls), -min(Ls)),
    )
    lb = -(-sum(counts) // NCORES)
    asg = None
    budget = [200000]  # global node budget across all candidates
    for Ls in cands:
        if sum(Ls) < lb:
            continue
        asg = _try_assign(Ls, tuple(counts), budget)
        if asg is not None:
            break
        if budget[0] < 0:
            break
    if asg is None:
        Ls, asg = _FALLBACK, _try_assign(_FALLBACK, tuple(counts))
        assert asg is not None
    # reorder classes descending by size
    perm = sorted(range(len(Ls)), key=lambda k: -Ls[k])
    Ls2 = tuple(Ls[k] for k in perm)
    asg2 = tuple(tuple(ns[k] for k in perm) for ns in asg)
    return Ls2, asg2


def _make_plan(counts):
    """Return (Ls, chunks) where chunks[core][slot] = (expert, t0, n)."""
    Ls, asg = _find_structure(tuple(int(c) for c in counts))
    S = len(Ls)
    offs = np.concatenate([[0], np.cumsum(counts)]).astype(np.int64)
    # per size class, list of (expert, t0, n)
    per_class = [[] for _ in range(S)]
    for e, ns in enumerate(asg):
        pos = int(offs[e])
        remaining = int(counts[e])
        # fill largest class chunks first
        for k in sorted(range(S), key=lambda k: -Ls[k]):
            for _ in range(ns[k]):
                take = min(remaining, Ls[k])
                per_class[k].append((e, pos, take))
                pos += take
                remaining -= take
        assert remaining == 0
    chunks = [[None] * S for _ in range(NCORES)]
    for k in range(S):
        cl = per_class[k]
        assert len(cl) <= NCORES
        for j in range(NCORES):
            chunks[j][k] = cl[j] if j < len(cl) else (-1, 0, 0)
    return Ls, chunks


@lru_cache(maxsize=4)
def _build_program(Ls):
    import concourse.bacc as bacc
    import concourse.tile as tile
    from concourse import mybir

    f32 = mybir.dt.float32
    bf16 = mybir.dt.bfloat16
    nc = bacc.Bacc("TRN2", target_bir_lowering=False, debug=False,
                   num_devices=NCORES, name="moe_experts")

    xt_d, w13_d, w2_d, out_d = [], [], [], []
    for s, L in enumerate(Ls):
        xt_d.append(nc.dram_tensor(f"xt{s}", (DC, P, L), bf16, kind="ExternalInput"))
        # w1 and w3 packed per hc: 4KB contiguous per partition line
        w13_d.append(nc.dram_tensor(f"w13r{s}", (HC, P, 2, DC, P), bf16,
                                    kind="ExternalInput"))
        w2_d.append(nc.dram_tensor(f"w2r{s}", (DC, P, HC, P), bf16,
                                   kind="ExternalInput"))
        out_d.append(nc.dram_tensor(f"out{s}", (DC, P, L), bf16,
                                    kind="ExternalOutput"))

    def nchunks(L):
        # PSUM-bank-aligned matmul column chunks: 512s then a trailing 256/384
        out, n0 = [], 0
        while L - n0 >= 512:
            out.append((n0, 512))
            n0 += 512
        if L - n0:
            assert L - n0 in (256, 320, 384, 448), L
            out.append((n0, L - n0))
        return out

    with tile.TileContext(nc) as tc:
        with (
            tc.tile_pool(name="xpool", bufs=2) as xpool,
            tc.tile_pool(name="hpool", bufs=1) as hpool,
            tc.tile_pool(name="wpool", bufs=8) as wpool,
            tc.tile_pool(name="w2pool", bufs=DC) as w2pool,
            tc.tile_pool(name="spool", bufs=2) as spool,
            tc.tile_pool(name="psum", bufs=2, space="PSUM") as psum,
        ):
            # two independent in-order HWDGE rings; alternate streams so
            # neither ring head-of-line-blocks the other's consumers
            qs = [nc.sync, nc.scalar]

            for s, L in enumerate(Ls):
                xt = xpool.tile([P, DC, L], bf16, tag="xt")
                hbuf = hpool.tile([P, HC, L], bf16, tag="h")
                w2ts = [w2pool.tile([P, HC, P], bf16, tag="w2", name=f"w2t{dc}")
                        for dc in range(DC)]
                for hc in range(HC):
                    w13t = wpool.tile([P, 2, DC, P], bf16, tag="w13")
                    if hc == 0:
                        # first matmul needs xt[dc=0] + w1[hc=0,dc=0] only:
                        # put them at the head of their (separate) rings,
                        # per-dc for the very first tile so the p1 chain
                        # starts after ~32KB instead of ~512KB
                        if s == 0:
                            for dc in range(DC):
                                qs[0].dma_start(w13t[:, 0, dc], w13_d[s].ap()[hc, :, 0, dc])
                                qs[1].dma_start(xt[:, dc, :], xt_d[s].ap()[dc])
                            qs[0].dma_start(w13t[:, 1], w13_d[s].ap()[hc, :, 1])
                        else:
                            qs[0].dma_start(w13t[:, 0], w13_d[s].ap()[hc, :, 0])
                            for dc in range(DC):
                                qs[dc % 2].dma_start(xt[:, dc, :], xt_d[s].ap()[dc])
                            qs[0].dma_start(w13t[:, 1], w13_d[s].ap()[hc, :, 1])
                    else:
                        qs[hc % 2].dma_start(w13t[:, 0], w13_d[s].ap()[hc, :, 0])
                        qs[(hc + 1) % 2].dma_start(w13t[:, 1], w13_d[s].ap()[hc, :, 1])
                    if 10 <= hc < 10 + DC:
                        # spread phase-B weight loads across late phase A
                        # (early phase A is DMA-tight while rings ramp)
                        dc2 = hc - 10
                        qs[(hc + 1) % 2].dma_start(w2ts[dc2][:], w2_d[s].ap()[dc2])
                    p1 = psum.tile([P, L], f32, tag="p1")
                    p3 = psum.tile([P, L], f32, tag="p3")
                    # dc outer / column-chunk inner: consecutive matmuls
                    # share the stationary weight tile (near-zero issue
                    # overhead; reordering to chunk-outer or interleaving
                    # chains measures strictly slower)
                    for pt, w in ((p1, 0), (p3, 1)):
                        for dc in range(DC):
                            for (n0, nsz) in nchunks(L):
                                nc.tensor.matmul(
                                    pt[:, n0:n0 + nsz],
                                    w13t[:, w, dc, :],
                                    xt[:, dc, n0:n0 + nsz],
                                    start=(dc == 0), stop=(dc == DC - 1),
                                )
                    stmp = spool.tile([P, L], f32, tag="stmp")
                    nc.scalar.activation(stmp[:], p1[:], mybir.ActivationFunctionType.Silu)
                    nc.vector.tensor_mul(out=hbuf[:, hc, :], in0=stmp[:], in1=p3[:])
                for dc in range(DC):
                    po = psum.tile([P, L], f32, tag="p1")
                    for hc in range(HC):
                        for (n0, nsz) in nchunks(L):
                            nc.tensor.matmul(
                                po[:, n0:n0 + nsz],
                                w2ts[dc][:, hc, :],
                                hbuf[:, hc, n0:n0 + nsz],
                                start=(hc == 0), stop=(hc == HC - 1),
                            )
                    ot = spool.tile([P, L], bf16, tag="ot")
                    for (n0, nsz) in nchunks(L):
                        nc.any.tensor_copy(out=ot[:, n0:n0 + nsz],
                                           in_=po[:, n0:n0 + nsz])
                        qs[dc % 2].dma_start(out_d[s].ap()[dc, :, n0:n0 + nsz],
                                             ot[:, n0:n0 + nsz])

    nc.compile()
    return nc


def _bf16():
    import ml_dtypes
    return ml_dtypes.bfloat16


def _permute_w13(w1, w3):  # 2x [H, D] -> [HC, P(k=d), 2, DC, P(m=h)]
    bf = _bf16()
    out = np.empty((HC, P, 2, DC, P), dtype=bf)
    out[:, :, 0] = w1.reshape(HC, P, DC, P).transpose(0, 3, 2, 1).astype(bf)
    out[:, :, 1] = w3.reshape(HC, P, DC, P).transpose(0, 3, 2, 1).astype(bf)
    return out


def _permute_w2(w):  # [D, H] -> [DC, P(k=h), HC, P(m=d)]
    return w.reshape(DC, P, HC, P).transpose(0, 3, 2, 1).astype(_bf16())


def kernel(x, w1, w2, w3, num_tokens_per_expert):
    from concourse.bass_utils import run_bass_kernel_spmd

    x = np.asarray(x, dtype=np.float32)
    w1 = np.asarray(w1, dtype=np.float32)
    w2 = np.asarray(w2, dtype=np.float32)
    w3 = np.asarray(w3, dtype=np.float32)
    counts = np.asarray(num_tokens_per_expert).astype(np.int64)

    Ls, chunks = _make_plan(counts)
    nc = _build_program(tuple(Ls))

    experts_used = sorted({e for row in chunks for (e, _, _) in row if e >= 0})
    if not experts_used:
        experts_used = [0]
    w13r = {e: _permute_w13(w1[e], w3[e]) for e in experts_used}
    w2r = {e: _permute_w2(w2[e]) for e in experts_used}
    e_dummy = experts_used[0]

    in_maps = []
    for c in range(NCORES):
        m = {}
        for s, L in enumerate(Ls):
            e, t0, n = chunks[c][s]
            if e < 0:
                e = e_dummy
            xs = np.zeros((L, D), dtype=np.float32)
            if n:
                xs[:n] = x[t0:t0 + n]
            m[f"xt{s}"] = xs.reshape(L, DC, P).transpose(1, 2, 0).astype(_bf16())
            m[f"w13r{s}"] = w13r[e]
            m[f"w2r{s}"] = w2r[e]
        in_maps.append(m)

    res = run_bass_kernel_spmd(nc, in_maps, core_ids=list(range(NCORES)))

    out = np.zeros((T, D), dtype=np.float32)
    for c in range(NCORES):
        for s in range(len(Ls)):
            e, t0, n = chunks[c][s]
            if e < 0 or n == 0:
                continue
            o = res.results[c][f"out{s}"]  # [DC, P, L] bf16
            out[t0:t0 + n] = (
                o[:, :, :n].transpose(2, 0, 1).reshape(n, D).astype(np.float32))
    return out
